# revision 1
# baseline (speedup 1.0000x reference)
"""Trainium2 Bass kernel for agent attention (sparse_attention problem).

Per-core work (data-parallel over batch B=8 across 8 NeuronCores):
  x[b] [256, 64, 64] -> qkv 3x3 conv (dif-conv + BN folded into weights)
  -> agent attention (8 heads, d=32, 64 agent tokens)
  -> depthwise 3x3 pe conv on v -> 1x1 proj.

v3: f32r conv matmuls (self-loading weights, no exposed LDWEIGHTS),
bf16 input staging for fast DMA, GpSimd depthwise pe conv, paired
stage-1 exps, restructured stage-2 (g = attnZ^T @ e2 full-width
matmuls, matmul-broadcast softmax denominator, fast reciprocal).
"""
import numpy as np

NUM_HEADS = 8
AGENT_NUM = 64
THETA = 0.7
C = 256
H = W = 64
HW = H * W
D = C // NUM_HEADS          # 32
N_AG = AGENT_NUM            # 64
PS = 8                      # pool size
N_CORES = 8
B = 8

_cache = {}


def _build():
    import concourse.bass as bass
    import concourse.tile as tile
    from concourse import bacc, mybir

    f32 = mybir.dt.float32
    f32r = mybir.dt.float32r
    bf16 = mybir.dt.bfloat16
    AF = mybir.ActivationFunctionType
    ALU = mybir.AluOpType
    AX = mybir.AxisListType

    nc = bacc.Bacc("TRN2", target_bir_lowering=False, debug=False,
                   enable_asserts=True, num_devices=N_CORES)

    X = nc.dram_tensor("x", [2, 128, H, W], bf16, kind="ExternalInput").ap()
    WQ = nc.dram_tensor("wq", [6, 128, 2, 9, 128], f32r,
                        kind="ExternalInput").ap()
    BQ = nc.dram_tensor("bq", [128, 6], f32, kind="ExternalInput").ap()
    PEW = nc.dram_tensor("pew", [128, 2, 9], f32, kind="ExternalInput").ap()
    PW = nc.dram_tensor("pw", [128, 2 * 256], f32r, kind="ExternalInput").ap()
    PB = nc.dram_tensor("pb", [128, 2], f32, kind="ExternalInput").ap()
    IDN = nc.dram_tensor("idn", [128, 128], bf16, kind="ExternalInput").ap()
    OUT = nc.dram_tensor("out", [2, 128, HW], f32, kind="ExternalOutput").ap()

    # softmax exp scale: d^-0.5, with the 1/64 agent-pool mean folded in
    SCALE = (D ** -0.5) / (PS * PS)

    with tile.TileContext(nc) as tc:
        from contextlib import ExitStack
        with ExitStack() as top:
            pers = top.enter_context(tc.tile_pool(name="pers", bufs=1))
            q_sb = [pers.tile([128, HW], f32r, tag=f"q{i}", name=f"q{i}")
                    for i in range(2)]
            k_sb = [pers.tile([128, HW], bf16, tag=f"k{i}", name=f"k{i}")
                    for i in range(2)]
            v_pad = [pers.tile([128, 66 * 66], bf16, tag=f"vp{i}",
                               name=f"vp{i}") for i in range(2)]
            att_out = [pers.tile([128, HW], f32r, tag=f"ao{i}", name=f"ao{i}")
                       for i in range(2)]
            bq = pers.tile([128, 6], f32, tag="bq", name="bq")
            idn = pers.tile([128, 128], bf16, tag="idn", name="idn")
            pew = pers.tile([128, 2, 9], f32, tag="pew", name="pew")
            asum_t = pers.tile([128, 128], f32, tag="asum", name="asum")
            a_sum = [asum_t[:, 64 * i:64 * (i + 1)] for i in range(2)]
            # a_bd4 needed in bf16 (stage-1 rhs) and f32r (stage-2 lhsT)
            abd_bf_t = pers.tile([128, 512], bf16, tag="abdb", name="abdb")
            abd_bf = [abd_bf_t[:, 256 * i:256 * (i + 1)] for i in range(2)]
            abd_f_t = pers.tile([128, 512], f32r, tag="abdf", name="abdf")
            az_t = pers.tile([128, 4 * 64], bf16, tag="az", name="az")
            attnZ = [az_t[:, 64 * i:64 * (i + 1)] for i in range(4)]
            pw = pers.tile([128, 2 * 256], f32r, tag="pw", name="pwt")
            pb = pers.tile([128, 2], f32, tag="pb", name="pbt")
            hsel = pers.tile([128, 64], bf16, tag="hsel", name="hsel")
            pwv = pw[:].rearrange("p (a b) -> p a b", a=2, b=256)

            # hsel: block-ones selector so hsel^T @ e2 replicates each
            # head's agent-sum across that head's 32 d-partitions
            nc.vector.memset(hsel[:], 0.0)
            nc.vector.memset(hsel[0:64, 0:32], 1.0)
            nc.vector.memset(hsel[64:128, 32:64], 1.0)

            for cc in range(2):
                vv = v_pad[cc][:].rearrange("p (r c) -> p r c", r=66, c=66)
                nc.vector.memset(vv[:, 0:1, :], 0.0)
                nc.vector.memset(vv[:, 65:66, :], 0.0)
                nc.vector.memset(vv[:, :, 0:1], 0.0)
                nc.vector.memset(vv[:, :, 65:66], 0.0)

            # persistent v^T chunk tiles (layout per cc-section of 130:
            # [64 ch of hp-even | 2 ones | 64 ch of hp-odd])
            s1sb = top.enter_context(tc.tile_pool(name="s1sb", bufs=1))
            vts = [s1sb.tile([128, 260], bf16, tag=f"vt{i}", name=f"vt{i}")
                   for i in range(32)]

            with ExitStack() as ph:
                tr_ps = ph.enter_context(
                    tc.tile_pool(name="trps", bufs=2, space="PSUM"))
                cpool = ph.enter_context(tc.tile_pool(name="conv", bufs=1))
                wpool = ph.enter_context(tc.tile_pool(name="wq", bufs=1))
                cps = ph.enter_context(
                    tc.tile_pool(name="cps", bufs=6, space="PSUM"))

                x_pad = [cpool.tile([128, 66 * 66], f32r, tag=f"xp{i}",
                                    name=f"xp{i}") for i in range(2)]
                x_stg = [cpool.tile([128, HW], bf16, tag=f"xs{i}",
                                    name=f"xs{i}") for i in range(2)]
                for kc in range(2):
                    xv = x_pad[kc][:].bitcast(f32).rearrange(
                        "p (r c) -> p r c", r=66, c=66)
                    nc.vector.memset(xv[:, 0:1, :], 0.0)
                    nc.vector.memset(xv[:, 65:66, :], 0.0)
                    nc.vector.memset(xv[:, :, 0:1], 0.0)
                    nc.vector.memset(xv[:, :, 65:66], 0.0)

                # first conv group's weights ahead of the bulk x transfer;
                # x lands as one fully-contiguous transfer per group and
                # DVE pads/casts it to f32
                wq4 = []
                for kc in range(2):
                    wt = wpool.tile([128, 9, 128], f32r, tag="w", name="w",
                                    bufs=4)
                    nc.sync.dma_start(wt[:], WQ[4, :, kc])
                    wq4.append(wt)
                for kc in range(2):
                    xsv = x_stg[kc][:].rearrange("p (r c) -> p r c",
                                                 r=64, c=64)
                    nc.sync.dma_start(xsv[:, 0:32, :], X[kc, :, 0:32, :])
                for kc in range(2):
                    xsv = x_stg[kc][:].rearrange("p (r c) -> p r c",
                                                 r=64, c=64)
                    nc.sync.dma_start(xsv[:, 32:64, :], X[kc, :, 32:64, :])
                nc.sync.dma_start(bq[:], BQ[:])
                nc.sync.dma_start(idn[:], IDN[:])
                nc.sync.dma_start(pew[:], PEW[:])
                nc.sync.dma_start(pw[:], PW[:])
                nc.sync.dma_start(pb[:], PB[:])
                for r0 in range(0, 64, 16):
                    for kc in range(2):
                        xv = x_pad[kc][:].rearrange(
                            "p (r c) -> p r c", r=66, c=66)
                        xs = x_stg[kc][:].rearrange("p (r c) -> p r c",
                                                    r=64, c=64)
                        nc.vector.tensor_copy(
                            xv[:, r0 + 1:r0 + 17, 1:65],
                            xs[:, r0:r0 + 16, :])

                def conv_group(mc, wts=None, extra=None):
                    if wts is None:
                        wts = []
                        for kc in range(2):
                            wt = wpool.tile([128, 9, 128], f32r, tag="w",
                                            name="w", bufs=4)
                            nc.sync.dma_start(wt[:], WQ[mc, :, kc])
                            wts.append(wt)
                    for rb in range(8):
                        ps_t = cps.tile([128, 512], f32, tag="cps",
                                        name="cpst")
                        psv = ps_t[:].rearrange("p (r c) -> p r c", r=8, c=64)
                        i = 0
                        for kc in range(2):
                            xv = x_pad[kc][:].rearrange(
                                "p (r c) -> p r c", r=66, c=66)
                            for s in range(9):
                                ky, kx = s // 3, s % 3
                                rhs = xv[:, 8 * rb + ky: 8 * rb + ky + 8,
                                         kx: kx + 64]
                                nc.tensor.matmul(
                                    psv, wts[kc][:, s, :], rhs,
                                    start=(i == 0), stop=(i == 17))
                                i += 1
                        bias = bq[:, mc: mc + 1]
                        if mc < 2:
                            dst = q_sb[mc][:, 512 * rb: 512 * (rb + 1)]
                            nc.scalar.add(dst, ps_t[:], bias)
                        elif mc < 4:
                            dst = k_sb[mc - 2][:, 512 * rb: 512 * (rb + 1)]
                            nc.scalar.add(dst, ps_t[:], bias)
                        else:
                            vv = v_pad[mc - 4][:].rearrange(
                                "p (r c) -> p r c", r=66, c=66)
                            dst = vv[:, 8 * rb + 1: 8 * rb + 9, 1:65]
                            nc.scalar.add(dst, psv, bias)
                        if extra is not None:
                            extra(rb)

                # transposed v chunk builder: DVE staging copy + one PE
                # transpose + 2 DVE copies per (ch, cc)
                def make_vt(ch):
                    vtc = vts[ch]
                    for cc in range(2):
                        vv = v_pad[cc][:].rearrange(
                            "p (r c) -> p r c", r=66, c=66)
                        vstg = wpool.tile([128, 128], bf16, tag="vstg",
                                          name="vstg", bufs=4)
                        nc.vector.tensor_copy(
                            vstg[:].rearrange("p (r c) -> p r c", r=2, c=64),
                            vv[:, 2 * ch + 1: 2 * ch + 3, 1:65])
                        tp = tr_ps.tile([128, 128], bf16, tag="tr",
                                        name="trt")
                        nc.tensor.transpose(tp[:], vstg[:], idn[:])
                        nc.vector.tensor_copy(
                            vtc[:, 130 * cc:130 * cc + 64], tp[:, 0:64])
                        nc.vector.tensor_copy(
                            vtc[:, 130 * cc + 66:130 * cc + 130],
                            tp[:, 64:128])

                # pe depthwise conv on DVE, accumulating into att_out
                def pe_conv(cc, g):
                    vvf = v_pad[cc][:].rearrange(
                        "p (r c) -> p r c", r=66, c=66)
                    aof = att_out[cc][:].rearrange(
                        "p (r c) -> p r c", r=64, c=64)
                    r0 = 16 * g
                    dst = aof[:, r0:r0 + 16, :]
                    for s in range(9):
                        ky, kx = s // 3, s % 3
                        sv = vvf[:, r0 + ky: r0 + ky + 16, kx: kx + 64]
                        if s == 0:
                            nc.vector.tensor_scalar_mul(
                                dst, sv, pew[:, cc, 0:1])
                        else:
                            nc.vector.scalar_tensor_tensor(
                                dst, sv, pew[:, cc, s:s + 1], dst,
                                ALU.mult, ALU.add)

                # v first
                conv_group(4, wts=wq4)
                for i in range(32):
                    for cc in range(2):
                        nc.vector.memset(
                            vts[i][:, 130 * cc + 64:130 * cc + 66], 1.0)
                conv_group(5)

                # during q/k conv: transposes spread 1 per rb, pe on GpSimd
                nvt = [0]

                def vt_extra(rb):
                    if nvt[0] < 32:
                        make_vt(nvt[0])
                        nvt[0] += 1

                conv_group(0, extra=vt_extra)
                qv0 = q_sb[0][:].rearrange(
                    "p (by dy bx dx) -> p by bx dy dx",
                    by=8, dy=8, bx=8, dx=8)
                nc.vector.tensor_reduce(a_sum[0], qv0, AX.XY, ALU.add)
                pe_conv(0, 0)
                pe_conv(0, 1)
                conv_group(1, extra=vt_extra)
                pe_conv(0, 2)
                pe_conv(0, 3)

                # pooling + block-diag a (both dtypes)
                qv1 = q_sb[1][:].rearrange(
                    "p (by dy bx dx) -> p by bx dy dx",
                    by=8, dy=8, bx=8, dx=8)
                nc.vector.tensor_reduce(a_sum[1], qv1, AX.XY, ALU.add)
                nc.vector.memset(abd_bf_t[:], 0.0)
                nc.vector.memset(abd_f_t[:].bitcast(f32), 0.0)
                for cc in range(2):
                    for j in range(4):
                        nc.vector.tensor_copy(
                            abd_bf[cc][32 * j:32 * j + 32,
                                       64 * j:64 * j + 64],
                            a_sum[cc][32 * j:32 * j + 32, :])
                        nc.vector.tensor_copy(
                            abd_f_t[32 * j:32 * j + 32,
                                    256 * cc + 64 * j:256 * cc + 64 * j + 64],
                            a_sum[cc][32 * j:32 * j + 32, :])

                # k
                conv_group(2, extra=vt_extra)
                pe_conv(1, 0)
                pe_conv(1, 1)
                conv_group(3, extra=vt_extra)
                pe_conv(1, 2)
                pe_conv(1, 3)
                while nvt[0] < 32:
                    make_vt(nvt[0])
                    nvt[0] += 1

            # ---- stage 1 ----
            # attn_ps[hp] accumulates [128 agents, 66] over 32 chunks:
            # for half 0 cols = [64 ch | Z Z], for half 1 cols = [Z Z | 64 ch]
            with ExitStack() as ph:
                st_ps = ph.enter_context(
                    tc.tile_pool(name="stps", bufs=4, space="PSUM"))
                at_ps = ph.enter_context(
                    tc.tile_pool(name="atps", bufs=4, space="PSUM"))
                etp = ph.enter_context(tc.tile_pool(name="etp", bufs=1))
                attn_ps = [at_ps.tile([128, 66], f32, tag="at", name="at")
                           for _ in range(4)]
                for chp in range(16):   # ch pairs
                    for cc in range(2):
                        sp = st_ps.tile([128, 512], f32, tag="st",
                                        name="stt")
                        for u in range(2):
                            ch = 2 * chp + u
                            nc.tensor.matmul(
                                sp[:, 256 * u:256 * (u + 1)],
                                k_sb[cc][:, 128 * ch:128 * (ch + 1)],
                                abd_bf[cc], start=True, stop=True,
                                skip_group_check=True)
                        et = etp.tile([128, 512], bf16, tag="et",
                                      name="et", bufs=6)
                        nc.scalar.activation(et[:], sp[:], AF.Exp,
                                             scale=SCALE)
                        for u in range(2):
                            ch = 2 * chp + u
                            for half in range(2):
                                hp = 2 * cc + half
                                rhs = vts[ch][:, 130 * cc + 64 * half:
                                              130 * cc + 64 * half + 66]
                                nc.tensor.matmul(
                                    attn_ps[hp][:],
                                    et[:, 256 * u + 128 * half:
                                       256 * u + 128 * (half + 1)],
                                    rhs, start=(ch == 0), stop=(ch == 31))

                # normalize stage-1 rows by Z1 -> attnZ [128 agents, 64]
                # (emitted after hoisted stage-2 score matmuls below)
                for hp in range(4):
                    half = hp % 2
                    zc = 64 if half == 0 else 0
                    och = 0 if half == 0 else 2
                    r1 = etp.tile([128, 1], f32, tag="r1", name="r1", bufs=4)
                    nc.vector.reciprocal(r1[:], attn_ps[hp][:, zc:zc + 1])
                    nc.vector.memset(attnZ[hp], 0.0)
                    nc.vector.tensor_scalar_mul(
                        attnZ[hp][0:64, 0:32],
                        attn_ps[hp][0:64, och:och + 32], r1[0:64, :])
                    nc.vector.tensor_scalar_mul(
                        attnZ[hp][64:128, 32:64],
                        attn_ps[hp][64:128, och + 32:och + 64],
                        r1[64:128, :])

            # ---- stage 2 + proj ----
            with ExitStack() as ph:
                s2sb = ph.enter_context(tc.tile_pool(name="s2sb", bufs=6))
                osb = ph.enter_context(tc.tile_pool(name="osb", bufs=3))
                s2_ps = ph.enter_context(
                    tc.tile_pool(name="s2ps", bufs=2, space="PSUM"))
                g_ps = ph.enter_context(
                    tc.tile_pool(name="gps", bufs=2, space="PSUM"))
                z_ps = ph.enter_context(
                    tc.tile_pool(name="zps", bufs=2, space="PSUM"))
                pr_ps = ph.enter_context(
                    tc.tile_pool(name="prps", bufs=2, space="PSUM"))

                for nt in range(8):
                    for cc in range(2):
                        gp = g_ps.tile([128, 512], f32, tag="g", name="gt")
                        zp = z_ps.tile([128, 512], f32, tag="z", name="zt")
                        for half in range(2):
                            hp = 2 * cc + half
                            sp = s2_ps.tile([128, 512], f32, tag="s2",
                                            name="s2t")
                            nc.tensor.matmul(
                                sp[:],
                                abd_f_t[:, 256 * cc + 128 * half:
                                        256 * cc + 128 * (half + 1)],
                                q_sb[cc][:, 512 * nt:512 * (nt + 1)],
                                start=True, stop=True)
                            e2 = s2sb.tile([128, 512], bf16, tag="e2",
                                           name="e2")
                            nc.scalar.activation(e2[:], sp[:], AF.Exp,
                                                 scale=SCALE)
                            # g rows 0:64 (half 0) / 64:128 (half 1)
                            nc.tensor.matmul(
                                gp[64 * half:64 * half + 64, :],
                                attnZ[hp], e2[:], start=True, stop=True,
                                skip_group_check=True)
                            # Zb rows: per-head agent sums of e2, already
                            # replicated to each head's 32 d-partitions
                            nc.tensor.matmul(
                                zp[64 * half:64 * half + 64, :],
                                hsel[:], e2[:], start=True, stop=True,
                                skip_group_check=True)
                        rb = s2sb.tile([128, 512], f32, tag="rb", name="rbt")
                        nc.vector.reciprocal_approx_fast(rb[:], zp[:])
                        tsc = s2sb.tile([128, 512], f32r, tag="ts",
                                        name="tsc")
                        nc.vector.tensor_tensor(tsc[:], gp[:], rb[:],
                                                ALU.mult)
                        sl = att_out[cc][:, 512 * nt:512 * (nt + 1)]
                        nc.vector.tensor_tensor(sl, tsc[:].bitcast(f32),
                                                sl.bitcast(f32), ALU.add)
                    for mc in range(2):
                        pp = pr_ps.tile([128, 512], f32, tag="tp", name="prt")
                        for kc in range(2):
                            nc.tensor.matmul(
                                pp[:], pwv[:, kc, 128 * mc:128 * (mc + 1)],
                                att_out[kc][:, 512 * nt:512 * (nt + 1)],
                                start=(kc == 0), stop=(kc == 1))
                        ot = osb.tile([128, 512], f32, tag="ot", name="ott")
                        nc.vector.tensor_scalar_add(ot[:], pp[:],
                                                    pb[:, mc:mc + 1])
                        nc.sync.dma_start(
                            OUT[mc, :, 512 * nt:512 * (nt + 1)], ot[:])

    nc.compile()
    return nc


def _prep_consts(qkv_w, qkv_s, qkv_b, pe_w, pe_s, pe_b, proj_w, proj_s,
                 proj_b):
    import ml_dtypes
    f = np.float32
    bf = ml_dtypes.bfloat16
    w = np.asarray(qkv_w, f).copy()          # [768, 256, 3, 3]
    dif = (w[:, :, 0, 1] + w[:, :, 1, 0] + w[:, :, 1, 1] + w[:, :, 1, 2]
           + w[:, :, 2, 1])
    w[:, :, 1, 1] -= THETA * dif
    w *= np.asarray(qkv_s, f)[:, None, None, None]
    # WQ[mc, p, kc, s, o'] = w[128*mc+o', 128*kc+p, s//3, s%3]
    wq = w.reshape(6, 128, 2, 128, 9)        # [mc, o', kc, p, s]
    wq = np.ascontiguousarray(wq.transpose(0, 3, 2, 4, 1))  # [6,128,2,9,128]

    bq = np.ascontiguousarray(np.asarray(qkv_b, f).reshape(6, 128).T)

    pe_wf = np.asarray(pe_w, f)[:, 0] * np.asarray(pe_s, f)[:, None, None]
    pew = np.zeros((128, 2, 9), f)
    for kc in range(2):
        for s in range(9):
            pew[:, kc, s] = pe_wf[128 * kc:128 * (kc + 1), s // 3, s % 3]

    pwm = np.asarray(proj_w, f)[:, :, 0, 0] * np.asarray(proj_s, f)[:, None]
    pw = np.ascontiguousarray(
        pwm.T.reshape(2, 128, 256).transpose(1, 0, 2).reshape(128, 512))
    pbv = np.asarray(proj_b, f) + pwm @ np.asarray(pe_b, f)
    pb = np.ascontiguousarray(pbv.reshape(2, 128).T)

    idn = np.eye(128, dtype=bf)
    return dict(wq=wq, bq=bq, pew=pew, pw=pw, pb=pb, idn=idn)


def kernel(x, qkv_w, qkv_s, qkv_b, pe_w, pe_s, pe_b, proj_w, proj_s, proj_b):
    import ml_dtypes
    from concourse.bass_utils import run_bass_kernel_spmd

    if "nc" not in _cache:
        _cache["nc"] = _build()
    nc = _cache["nc"]

    consts = _prep_consts(qkv_w, qkv_s, qkv_b, pe_w, pe_s, pe_b, proj_w,
                          proj_s, proj_b)
    x = np.asarray(x, np.float32).astype(ml_dtypes.bfloat16)
    in_maps = []
    for b in range(B):
        m = dict(consts)
        m["x"] = np.ascontiguousarray(x[b].reshape(2, 128, H, W))
        in_maps.append(m)

    res = run_bass_kernel_spmd(nc, in_maps, list(range(N_CORES)), trace=False)
    out = np.empty((B, C, H, W), np.float32)
    for b in range(B):
        out[b] = res.results[b]["out"].reshape(C, H, W)
    return out



# revision 3
# speedup vs baseline: 1.1205x; 1.1205x over previous
"""Trainium2 Bass kernel for agent attention (sparse_attention problem).

Per-core work (data-parallel over batch B=8 across 8 NeuronCores):
  x[b] [256, 64, 64] -> qkv 3x3 conv (dif-conv + BN folded into weights)
  -> agent attention (8 heads, d=32, 64 agent tokens)
  -> depthwise 3x3 pe conv on v -> 1x1 proj.

v4: all-bf16 datapath (bf16 conv operands enable FWL weight loads),
unpadded x/v with edge-restricted conv access patterns (center tap
first so every PSUM element is initialized before partial-coverage
taps accumulate), v^T built with 4 batched XBAR DMA transposes instead
of 64 PE transposes, bf16 output DMA.
"""
import numpy as np

NUM_HEADS = 8
AGENT_NUM = 64
THETA = 0.7
C = 256
H = W = 64
HW = H * W
D = C // NUM_HEADS          # 32
N_CORES = 8
B = 8
PS = 8

_cache = {}


def _build():
    import concourse.bass as bass
    import concourse.tile as tile
    from concourse import bacc, mybir

    f32 = mybir.dt.float32
    bf16 = mybir.dt.bfloat16
    AF = mybir.ActivationFunctionType
    ALU = mybir.AluOpType
    AX = mybir.AxisListType

    nc = bacc.Bacc("TRN2", target_bir_lowering=False, debug=False,
                   enable_asserts=True, num_devices=N_CORES)

    X = nc.dram_tensor("x", [2, 128, HW], bf16, kind="ExternalInput").ap()
    # WQ[mc, p, kc, s, o'] = w[128*mc+o', 128*kc+p, s//3, s%3]
    WQ = nc.dram_tensor("wq", [6, 128, 2, 9, 128], bf16,
                        kind="ExternalInput").ap()
    BQ = nc.dram_tensor("bq", [128, 6], f32, kind="ExternalInput").ap()
    PEW = nc.dram_tensor("pew", [128, 2, 9], f32, kind="ExternalInput").ap()
    PW = nc.dram_tensor("pw", [128, 2 * 256], bf16, kind="ExternalInput").ap()
    PB = nc.dram_tensor("pb", [128, 2], f32, kind="ExternalInput").ap()
    OUT = nc.dram_tensor("out", [2, 128, HW], bf16, kind="ExternalOutput").ap()

    # softmax exp scale: d^-0.5, with the 1/64 agent-pool mean folded in
    SCALE = (D ** -0.5) / (PS * PS)

    # conv tap order: center tap first (full coverage) so partial taps
    # can accumulate onto initialized PSUM
    TAPS = [(0, 4), (1, 4)] + [(kc, s) for s in (0, 1, 2, 3, 5, 6, 7, 8)
                               for kc in (0, 1)]

    with tile.TileContext(nc) as tc:
        from contextlib import ExitStack
        with ExitStack() as top:
            pers = top.enter_context(tc.tile_pool(name="pers", bufs=1))
            x_sb = [pers.tile([128, HW], bf16, tag=f"x{i}", name=f"x{i}")
                    for i in range(2)]
            q_sb = [pers.tile([128, HW], bf16, tag=f"q{i}", name=f"q{i}")
                    for i in range(2)]
            k_sb = [pers.tile([128, HW], bf16, tag=f"k{i}", name=f"k{i}")
                    for i in range(2)]
            v_sb = [pers.tile([128, HW], bf16, tag=f"v{i}", name=f"v{i}")
                    for i in range(2)]
            att_out = [pers.tile([128, HW], bf16, tag=f"ao{i}", name=f"ao{i}")
                       for i in range(2)]
            wq_all = pers.tile([128, 6, 2, 9, 128], bf16, tag="wq", name="wq")
            bq = pers.tile([128, 6], f32, tag="bq", name="bq")
            pew = pers.tile([128, 2, 9], f32, tag="pew", name="pew")
            asum_t = pers.tile([128, 128], f32, tag="asum", name="asum")
            a_sum = [asum_t[:, 64 * i:64 * (i + 1)] for i in range(2)]
            # block-diag agent sums: stage-1 rhs and stage-2 lhsT
            abd_t = pers.tile([128, 512], bf16, tag="abd", name="abd")
            abd = [abd_t[:, 256 * i:256 * (i + 1)] for i in range(2)]
            az_t = pers.tile([128, 4 * 64], bf16, tag="az", name="az")
            attnZ = [az_t[:, 64 * i:64 * (i + 1)] for i in range(4)]
            pw = pers.tile([128, 2 * 256], bf16, tag="pw", name="pwt")
            pb = pers.tile([128, 2], f32, tag="pb", name="pbt")
            hsel = pers.tile([128, 64], bf16, tag="hsel", name="hsel")
            # v^T: per 128-px chunk ch, section cc holds
            # [64 ch of part 0:64 | 2 ones | 64 ch of part 64:128]
            vts_t = pers.tile([128, 32, 260], bf16, tag="vts", name="vts")
            pwv = pw[:].rearrange("p (a b) -> p a b", a=2, b=256)

            # hsel: block-ones selector so hsel^T @ e2 replicates each
            # head's agent-sum across that head's 32 d-partitions
            nc.vector.memset(hsel[:], 0.0)
            nc.vector.memset(hsel[0:64, 0:32], 1.0)
            nc.vector.memset(hsel[64:128, 32:64], 1.0)
            nc.vector.memset(vts_t[:, :, 64:66], 1.0)
            nc.vector.memset(vts_t[:, :, 194:196], 1.0)

            # weights first (mc=4/5 feed the first conv groups), x next
            for mc in (4, 5, 0, 1, 2, 3):
                nc.sync.dma_start(wq_all[:, mc], WQ[mc])
            for kc in range(2):
                nc.sync.dma_start(x_sb[kc][:, 0:2048], X[kc, :, 0:2048])
                nc.sync.dma_start(x_sb[kc][:, 2048:4096], X[kc, :, 2048:4096])
            nc.sync.dma_start(bq[:], BQ[:])
            nc.sync.dma_start(pew[:], PEW[:])
            nc.sync.dma_start(pw[:], PW[:])
            nc.sync.dma_start(pb[:], PB[:])

            with ExitStack() as ph:
                cps = ph.enter_context(
                    tc.tile_pool(name="cps", bufs=8, space="PSUM"))
                vtt = ph.enter_context(tc.tile_pool(name="vtt", bufs=2))

                def conv_group(mc):
                    for rb in range(8):
                        ps_t = cps.tile([128, 512], f32, tag="cps",
                                        name="cpst")
                        psv = ps_t[:].rearrange("p (r c) -> p r c", r=8, c=64)
                        for i, (kc, s) in enumerate(TAPS):
                            dy, dx = s // 3 - 1, s % 3 - 1
                            r_lo = 1 if (rb == 0 and dy == -1) else 0
                            r_hi = 7 if (rb == 7 and dy == 1) else 8
                            c_lo = 1 if dx == -1 else 0
                            c_hi = 63 if dx == 1 else 64
                            xv = x_sb[kc][:].rearrange(
                                "p (r c) -> p r c", r=64, c=64)
                            rhs = xv[:, 8 * rb + r_lo + dy:8 * rb + r_hi + dy,
                                     c_lo + dx:c_hi + dx]
                            nc.tensor.matmul(
                                psv[:, r_lo:r_hi, c_lo:c_hi],
                                wq_all[:, mc, kc, s, :], rhs,
                                start=(i == 0), stop=(i == 17),
                                skip_group_check=True)
                        bias = bq[:, mc:mc + 1]
                        if mc < 2:
                            dst = q_sb[mc][:, 512 * rb:512 * (rb + 1)]
                        elif mc < 4:
                            dst = k_sb[mc - 2][:, 512 * rb:512 * (rb + 1)]
                        else:
                            dst = v_sb[mc - 4][:, 512 * rb:512 * (rb + 1)]
                        nc.scalar.add(dst, ps_t[:], bias)

                # pe depthwise conv on DVE (bf16 2x), accumulating into
                # att_out; center tap first (full coverage mult), then
                # edge-restricted partial adds
                def pe_conv(cc, g):
                    vvf = v_sb[cc][:].rearrange("p (r c) -> p r c",
                                                r=64, c=64)
                    aof = att_out[cc][:].rearrange("p (r c) -> p r c",
                                                   r=64, c=64)
                    g0, g1 = 16 * g, 16 * g + 16
                    nc.vector.tensor_scalar_mul(
                        aof[:, g0:g1, :], vvf[:, g0:g1, :],
                        pew[:, cc, 4:5])
                    for s in (0, 1, 2, 3, 5, 6, 7, 8):
                        dy, dx = s // 3 - 1, s % 3 - 1
                        r_lo = max(g0, -dy)
                        r_hi = min(g1, 64 - dy)
                        c_lo = max(0, -dx)
                        c_hi = min(64, 64 - dx)
                        dst = aof[:, r_lo:r_hi, c_lo:c_hi]
                        src = vvf[:, r_lo + dy:r_hi + dy, c_lo + dx:c_hi + dx]
                        nc.vector.scalar_tensor_tensor(
                            dst, src, pew[:, cc, s:s + 1], dst,
                            ALU.mult, ALU.add)

                # v first
                conv_group(4)
                conv_group(5)

                # v^T via 4 batched XBAR DMA transposes + DVE copies into
                # the interleaved [64ch | ones | 64ch] layout
                for cc in range(2):
                    for half in range(2):
                        vt_stg = vtt.tile([128, 32, 64], bf16, tag="vstg",
                                          name="vstg")
                        nc.sync.dma_start_transpose(
                            vt_stg[:],
                            v_sb[cc][64 * half:64 * (half + 1), :])
                        nc.vector.tensor_copy(
                            vts_t[:, :, 130 * cc + 66 * half:
                                  130 * cc + 66 * half + 64],
                            vt_stg[:])

                conv_group(0)
                qv0 = q_sb[0][:].rearrange(
                    "p (by dy bx dx) -> p by bx dy dx",
                    by=8, dy=8, bx=8, dx=8)
                nc.vector.tensor_reduce(a_sum[0], qv0, AX.XY, ALU.add)
                pe_conv(0, 0)
                pe_conv(0, 1)
                conv_group(1)
                pe_conv(0, 2)
                pe_conv(0, 3)

                # pooling + block-diag a
                qv1 = q_sb[1][:].rearrange(
                    "p (by dy bx dx) -> p by bx dy dx",
                    by=8, dy=8, bx=8, dx=8)
                nc.vector.tensor_reduce(a_sum[1], qv1, AX.XY, ALU.add)
                nc.vector.memset(abd_t[:], 0.0)
                for cc in range(2):
                    for j in range(4):
                        nc.vector.tensor_copy(
                            abd[cc][32 * j:32 * j + 32, 64 * j:64 * j + 64],
                            a_sum[cc][32 * j:32 * j + 32, :])

                conv_group(2)
                pe_conv(1, 0)
                pe_conv(1, 1)
                conv_group(3)
                pe_conv(1, 2)
                pe_conv(1, 3)

            # ---- stage 1 ----
            # attn_ps[hp] accumulates [128 agents, 66] over 32 chunks:
            # for half 0 cols = [64 ch | Z Z], for half 1 cols = [Z Z | 64 ch]
            with ExitStack() as ph:
                st_ps = ph.enter_context(
                    tc.tile_pool(name="stps", bufs=4, space="PSUM"))
                at_ps = ph.enter_context(
                    tc.tile_pool(name="atps", bufs=4, space="PSUM"))
                etp = ph.enter_context(tc.tile_pool(name="etp", bufs=1))
                attn_ps = [at_ps.tile([128, 66], f32, tag="at", name="at")
                           for _ in range(4)]
                for chp in range(16):   # ch pairs
                    for cc in range(2):
                        sp = st_ps.tile([128, 512], f32, tag="st",
                                        name="stt")
                        for u in range(2):
                            ch = 2 * chp + u
                            nc.tensor.matmul(
                                sp[:, 256 * u:256 * (u + 1)],
                                k_sb[cc][:, 128 * ch:128 * (ch + 1)],
                                abd[cc], start=True, stop=True,
                                skip_group_check=True)
                        et = etp.tile([128, 512], bf16, tag="et",
                                      name="et", bufs=6)
                        nc.scalar.activation(et[:], sp[:], AF.Exp,
                                             scale=SCALE)
                        for u in range(2):
                            ch = 2 * chp + u
                            for half in range(2):
                                hp = 2 * cc + half
                                rhs = vts_t[:, ch, 130 * cc + 64 * half:
                                            130 * cc + 64 * half + 66]
                                nc.tensor.matmul(
                                    attn_ps[hp][:],
                                    et[:, 256 * u + 128 * half:
                                       256 * u + 128 * (half + 1)],
                                    rhs, start=(ch == 0), stop=(ch == 31))

                # normalize stage-1 rows by Z1 -> attnZ [128 agents, 64]
                for hp in range(4):
                    half = hp % 2
                    zc = 64 if half == 0 else 0
                    och = 0 if half == 0 else 2
                    r1 = etp.tile([128, 1], f32, tag="r1", name="r1", bufs=4)
                    nc.vector.reciprocal(r1[:], attn_ps[hp][:, zc:zc + 1])
                    nc.vector.memset(attnZ[hp], 0.0)
                    nc.vector.tensor_scalar_mul(
                        attnZ[hp][0:64, 0:32],
                        attn_ps[hp][0:64, och:och + 32], r1[0:64, :])
                    nc.vector.tensor_scalar_mul(
                        attnZ[hp][64:128, 32:64],
                        attn_ps[hp][64:128, och + 32:och + 64],
                        r1[64:128, :])

            # ---- stage 2 + proj ----
            with ExitStack() as ph:
                s2sb = ph.enter_context(tc.tile_pool(name="s2sb", bufs=6))
                osb = ph.enter_context(tc.tile_pool(name="osb", bufs=3))
                s2_ps = ph.enter_context(
                    tc.tile_pool(name="s2ps", bufs=2, space="PSUM"))
                g_ps = ph.enter_context(
                    tc.tile_pool(name="gps", bufs=2, space="PSUM"))
                z_ps = ph.enter_context(
                    tc.tile_pool(name="zps", bufs=2, space="PSUM"))
                pr_ps = ph.enter_context(
                    tc.tile_pool(name="prps", bufs=2, space="PSUM"))

                for nt in range(8):
                    for cc in range(2):
                        gp = g_ps.tile([128, 512], f32, tag="g", name="gt")
                        zp = z_ps.tile([128, 512], f32, tag="z", name="zt")
                        for half in range(2):
                            hp = 2 * cc + half
                            sp = s2_ps.tile([128, 512], f32, tag="s2",
                                            name="s2t")
                            nc.tensor.matmul(
                                sp[:],
                                abd_t[:, 256 * cc + 128 * half:
                                      256 * cc + 128 * (half + 1)],
                                q_sb[cc][:, 512 * nt:512 * (nt + 1)],
                                start=True, stop=True)
                            e2 = s2sb.tile([128, 512], bf16, tag="e2",
                                           name="e2")
                            nc.scalar.activation(e2[:], sp[:], AF.Exp,
                                                 scale=SCALE)
                            # g rows 0:64 (half 0) / 64:128 (half 1)
                            nc.tensor.matmul(
                                gp[64 * half:64 * half + 64, :],
                                attnZ[hp], e2[:], start=True, stop=True,
                                skip_group_check=True)
                            # Zb rows: per-head agent sums of e2, already
                            # replicated to each head's 32 d-partitions
                            nc.tensor.matmul(
                                zp[64 * half:64 * half + 64, :],
                                hsel[:], e2[:], start=True, stop=True,
                                skip_group_check=True)
                        rb = s2sb.tile([128, 512], f32, tag="rb", name="rbt")
                        nc.vector.reciprocal_approx_fast(rb[:], zp[:])
                        tsc = s2sb.tile([128, 512], bf16, tag="ts",
                                        name="tsc")
                        nc.vector.tensor_tensor(tsc[:], gp[:], rb[:],
                                                ALU.mult)
                        sl = att_out[cc][:, 512 * nt:512 * (nt + 1)]
                        nc.vector.tensor_tensor(sl, tsc[:], sl, ALU.add)
                    for mc in range(2):
                        pp = pr_ps.tile([128, 512], f32, tag="tp", name="prt")
                        for kc in range(2):
                            nc.tensor.matmul(
                                pp[:], pwv[:, kc, 128 * mc:128 * (mc + 1)],
                                att_out[kc][:, 512 * nt:512 * (nt + 1)],
                                start=(kc == 0), stop=(kc == 1))
                        ot = osb.tile([128, 512], bf16, tag="ot", name="ott")
                        nc.vector.tensor_scalar_add(ot[:], pp[:],
                                                    pb[:, mc:mc + 1])
                        nc.sync.dma_start(
                            OUT[mc, :, 512 * nt:512 * (nt + 1)], ot[:])

    nc.compile()
    return nc


def _prep_consts(qkv_w, qkv_s, qkv_b, pe_w, pe_s, pe_b, proj_w, proj_s,
                 proj_b):
    import ml_dtypes
    f = np.float32
    bf = ml_dtypes.bfloat16
    w = np.asarray(qkv_w, f).copy()          # [768, 256, 3, 3]
    dif = (w[:, :, 0, 1] + w[:, :, 1, 0] + w[:, :, 1, 1] + w[:, :, 1, 2]
           + w[:, :, 2, 1])
    w[:, :, 1, 1] -= THETA * dif
    w *= np.asarray(qkv_s, f)[:, None, None, None]
    # WQ[mc, p, kc, s, o'] = w[128*mc+o', 128*kc+p, s//3, s%3]
    wq = w.reshape(6, 128, 2, 128, 9)        # [mc, o', kc, p, s]
    wq = np.ascontiguousarray(
        wq.transpose(0, 3, 2, 4, 1)).astype(bf)   # [6,128,2,9,128]

    bq = np.ascontiguousarray(np.asarray(qkv_b, f).reshape(6, 128).T)

    pe_wf = np.asarray(pe_w, f)[:, 0] * np.asarray(pe_s, f)[:, None, None]
    pew = np.zeros((128, 2, 9), f)
    for kc in range(2):
        for s in range(9):
            pew[:, kc, s] = pe_wf[128 * kc:128 * (kc + 1), s // 3, s % 3]

    pwm = np.asarray(proj_w, f)[:, :, 0, 0] * np.asarray(proj_s, f)[:, None]
    pw = np.ascontiguousarray(
        pwm.T.reshape(2, 128, 256).transpose(1, 0, 2).reshape(
            128, 512)).astype(bf)
    pbv = np.asarray(proj_b, f) + pwm @ np.asarray(pe_b, f)
    pb = np.ascontiguousarray(pbv.reshape(2, 128).T)

    return dict(wq=wq, bq=bq, pew=pew, pw=pw, pb=pb)


def kernel(x, qkv_w, qkv_s, qkv_b, pe_w, pe_s, pe_b, proj_w, proj_s, proj_b):
    import ml_dtypes
    from concourse.bass_utils import run_bass_kernel_spmd

    if "nc" not in _cache:
        _cache["nc"] = _build()
    nc = _cache["nc"]

    consts = _prep_consts(qkv_w, qkv_s, qkv_b, pe_w, pe_s, pe_b, proj_w,
                          proj_s, proj_b)
    x = np.asarray(x, np.float32).astype(ml_dtypes.bfloat16)
    in_maps = []
    for b in range(B):
        m = dict(consts)
        m["x"] = np.ascontiguousarray(x[b].reshape(2, 128, HW))
        in_maps.append(m)

    res = run_bass_kernel_spmd(nc, in_maps, list(range(N_CORES)), trace=False)
    out = np.empty((B, C, H, W), np.float32)
    for b in range(B):
        out[b] = np.asarray(res.results[b]["out"],
                            np.float32).reshape(C, H, W)
    return out


# revision 8
# speedup vs baseline: 1.2348x; 1.1020x over previous
"""Trainium2 Bass kernel for agent attention (sparse_attention problem).

Per-core work (data-parallel over batch B=8 across 8 NeuronCores):
  x[b] [256, 64, 64] -> qkv 3x3 conv (dif-conv + BN folded into weights)
  -> agent attention (8 heads, d=32, 64 agent tokens)
  -> depthwise 3x3 pe conv on v -> 1x1 proj.

v5: 1-D Winograd F(2,3) along W for the qkv conv (6 MACs/output instead
of 9): DVE builds 4 transformed input planes U[j] (even/odd column
combinations, padded rows), PE accumulates P[j] = sum_{ky,kc}
What[j,ky,kc]^T U[j] per 16-row chunk into 4 PSUM banks, and a DVE
"step-2" combines P0..P3 into the two output column parities with the
bias folded in (writes q/k/v directly - no separate evacuation).
q/k/v/att_out live in a column-parity-interleaved layout [oc, b, r, q];
all downstream consumers use matching access patterns, and the proj
epilogue re-interleaves pixels on GpSimd before contiguous output DMA.
Depthwise pe conv runs on GpSimd. v^T via batched XBAR DMA transposes.
"""
import numpy as np

NUM_HEADS = 8
AGENT_NUM = 64
THETA = 0.7
C = 256
H = W = 64
HW = H * W
D = C // NUM_HEADS          # 32
N_CORES = 8
B = 8
PS = 8

_cache = {}


def _build():
    import concourse.bass as bass
    import concourse.tile as tile
    from concourse import bacc, mybir

    f32 = mybir.dt.float32
    bf16 = mybir.dt.bfloat16
    AF = mybir.ActivationFunctionType
    ALU = mybir.AluOpType
    AX = mybir.AxisListType

    nc = bacc.Bacc("TRN2", target_bir_lowering=False, debug=False,
                   enable_asserts=True, num_devices=N_CORES)

    X = nc.dram_tensor("x", [2, 128, HW], bf16, kind="ExternalInput").ap()
    # WQW[mc, p, j, ky, kc, o'] = What[j, ky][128*mc+o', 128*kc+p]
    WQW = nc.dram_tensor("wqw", [6, 128, 4, 3, 2, 128], bf16,
                         kind="ExternalInput").ap()
    BQ = nc.dram_tensor("bq", [128, 6], f32, kind="ExternalInput").ap()
    PEW = nc.dram_tensor("pew", [128, 2, 9], f32, kind="ExternalInput").ap()
    PW = nc.dram_tensor("pw", [128, 2 * 256], bf16, kind="ExternalInput").ap()
    PB = nc.dram_tensor("pb", [128, 2], f32, kind="ExternalInput").ap()
    OUT = nc.dram_tensor("out", [2, 128, HW], bf16, kind="ExternalOutput").ap()

    # softmax exp scale: d^-0.5, with the 1/64 agent-pool mean folded in
    SCALE = (D ** -0.5) / (PS * PS)

    with tile.TileContext(nc) as tc:
        from contextlib import ExitStack
        with ExitStack() as top:
            pers = top.enter_context(tc.tile_pool(name="pers", bufs=1))
            x_sb = [pers.tile([128, HW], bf16, tag=f"x{i}", name=f"x{i}")
                    for i in range(2)]
            # shuffled layout [oc, b(2), r(64), q(32)]: pixel (r, 2q+b)
            q_sb = [pers.tile([128, HW], bf16, tag=f"q{i}", name=f"q{i}")
                    for i in range(2)]
            k_sb = [pers.tile([128, HW], bf16, tag=f"k{i}", name=f"k{i}")
                    for i in range(2)]
            v_sb = [pers.tile([128, HW], bf16, tag=f"v{i}", name=f"v{i}")
                    for i in range(2)]
            att_out = [pers.tile([128, HW], bf16, tag=f"ao{i}", name=f"ao{i}")
                       for i in range(2)]
            # U[j][kc]: transformed input planes [128, 66 rows, 32 q]
            u_t = [[pers.tile([128, 66, 32], bf16, tag=f"u{j}{kc}",
                              name=f"u{j}{kc}") for kc in range(2)]
                   for j in range(4)]
            wq_all = pers.tile([128, 6, 4, 3, 2, 128], bf16, tag="wq",
                               name="wq")
            bq = pers.tile([128, 6], f32, tag="bq", name="bq")
            pew = pers.tile([128, 2, 9], f32, tag="pew", name="pew")
            asum_t = pers.tile([128, 128], f32, tag="asum", name="asum")
            a_sum = [asum_t[:, 64 * i:64 * (i + 1)] for i in range(2)]
            abd_t = pers.tile([128, 512], bf16, tag="abd", name="abd")
            abd = [abd_t[:, 256 * i:256 * (i + 1)] for i in range(2)]
            az_t = pers.tile([128, 4 * 64], bf16, tag="az", name="az")
            attnZ = [az_t[:, 64 * i:64 * (i + 1)] for i in range(4)]
            pw = pers.tile([128, 2 * 256], bf16, tag="pw", name="pwt")
            pb = pers.tile([128, 2], f32, tag="pb", name="pbt")
            hsel = pers.tile([128, 64], bf16, tag="hsel", name="hsel")
            vts_t = pers.tile([128, 32, 260], bf16, tag="vts", name="vts")
            pwv = pw[:].rearrange("p (a b) -> p a b", a=2, b=256)

            nc.vector.memset(hsel[:], 0.0)
            nc.vector.memset(hsel[0:64, 0:32], 1.0)
            nc.vector.memset(hsel[64:128, 32:64], 1.0)
            nc.vector.memset(vts_t[:, :, 64:66], 1.0)
            nc.vector.memset(vts_t[:, :, 194:196], 1.0)
            for j in range(4):
                for kc in range(2):
                    nc.vector.memset(u_t[j][kc][:, 0:1, :], 0.0)
                    nc.vector.memset(u_t[j][kc][:, 65:66, :], 0.0)

            # weights for the first conv groups, then x, then the rest
            nc.sync.dma_start(wq_all[:, 4], WQW[4])
            for kc in range(2):
                nc.sync.dma_start(x_sb[kc][:, 0:2048], X[kc, :, 0:2048])
                nc.sync.dma_start(x_sb[kc][:, 2048:4096], X[kc, :, 2048:4096])
            for mc in (5, 0, 1, 2, 3):
                nc.sync.dma_start(wq_all[:, mc], WQW[mc])
            nc.sync.dma_start(bq[:], BQ[:])
            nc.sync.dma_start(pew[:], PEW[:])
            nc.sync.dma_start(pw[:], PW[:])
            nc.sync.dma_start(pb[:], PB[:])

            # ---- input transform: U[j][kc][:, 1+r, q] over x rows r ----
            # xe = x[r, 2q], xo = x[r, 2q+1]
            # U0 = xo[q-1] - xo[q]  (q=0: -xo[0])
            # U1 = xe[q] + xo[q]
            # U2 = xo[q] - xe[q]
            # U3 = xe[q] - xe[q+1]  (q=31: xe[31])
            for kc in range(2):
                xv = x_sb[kc][:].rearrange("p (r q two) -> p r two q",
                                           r=64, q=32, two=2)
                xe = xv[:, :, 0:1, :]
                xo = xv[:, :, 1:2, :]
                for j in range(4):
                    uv = u_t[j][kc][:].rearrange("p r (one q) -> p r one q",
                                                 one=1, q=32)[:, 1:65]
                    if j == 0:
                        nc.vector.tensor_tensor(
                            uv[:, :, :, 1:32], xo[:, :, :, 0:31],
                            xo[:, :, :, 1:32], ALU.subtract)
                        nc.vector.tensor_scalar_mul(
                            uv[:, :, :, 0:1], xo[:, :, :, 0:1], -1.0)
                    elif j == 1:
                        nc.vector.tensor_tensor(
                            uv, xe, xo, ALU.add)
                    elif j == 2:
                        nc.vector.tensor_tensor(
                            uv, xo, xe, ALU.subtract)
                    else:
                        nc.vector.tensor_tensor(
                            uv[:, :, :, 0:31], xe[:, :, :, 0:31],
                            xe[:, :, :, 1:32], ALU.subtract)
                        nc.vector.tensor_copy(
                            uv[:, :, :, 31:32], xe[:, :, :, 31:32])

            with ExitStack() as ph:
                cps = ph.enter_context(
                    tc.tile_pool(name="cps", bufs=8, space="PSUM"))
                vtt = ph.enter_context(tc.tile_pool(name="vtt", bufs=2))
                stp = ph.enter_context(tc.tile_pool(name="stp", bufs=4))

                # one conv output group: 128 out-channels (mc), shuffled
                # layout; 4 chunks of 16 rows; P[j] accumulated on PE,
                # combined + biased on DVE, written straight to q/k/v
                def conv_group(mc):
                    if mc < 2:
                        dst_t = q_sb[mc]
                    elif mc < 4:
                        dst_t = k_sb[mc - 2]
                    else:
                        dst_t = v_sb[mc - 4]
                    bias = bq[:, mc:mc + 1]
                    for c in range(4):
                        ps = []
                        for j in range(4):
                            p_t = cps.tile([128, 512], f32, tag="cps",
                                           name="cpst")
                            for i, (ky, kc) in enumerate(
                                    (ky, kc) for ky in range(3)
                                    for kc in range(2)):
                                rhs = u_t[j][kc][:, 16 * c + ky:
                                                 16 * c + ky + 16, :]
                                nc.tensor.matmul(
                                    p_t[:], wq_all[:, mc, j, ky, kc, :],
                                    rhs, start=(i == 0), stop=(i == 5))
                            ps.append(p_t)
                        # step-2: b=0 -> P0+P1+P2+bias ; b=1 -> P1-P2-P3+bias
                        # ACT evacuates the four PSUM banks to SBUF bf16
                        # (folding the bias and the P3 sign), then all-SBUF
                        # TT chains run on DVE (b=0) and GpSimd (b=1)
                        p0b = stp.tile([128, 512], bf16, tag="p0b",
                                       name="p0b", bufs=2)
                        nc.scalar.add(p0b[:], ps[0][:], bias)
                        p1e = stp.tile([128, 512], bf16, tag="p1e",
                                       name="p1e", bufs=2)
                        nc.scalar.copy(p1e[:], ps[1][:])
                        p2e = stp.tile([128, 512], bf16, tag="p2e",
                                       name="p2e", bufs=2)
                        nc.scalar.copy(p2e[:], ps[2][:])
                        p3n = stp.tile([128, 512], bf16, tag="p3n",
                                       name="p3n", bufs=2)
                        nc.scalar.activation(p3n[:], ps[3][:], AF.Identity,
                                             bias=bias, scale=-1.0)
                        t0b = stp.tile([128, 512], bf16, tag="t0b",
                                       name="t0b", bufs=2)
                        nc.vector.tensor_tensor(t0b[:], p0b[:], p1e[:],
                                                ALU.add)
                        nc.vector.tensor_tensor(
                            dst_t[:, 512 * c:512 * (c + 1)], t0b[:],
                            p2e[:], ALU.add)
                        t1b = stp.tile([128, 512], bf16, tag="t1b",
                                       name="t1b", bufs=2)
                        nc.gpsimd.tensor_tensor(t1b[:], p1e[:], p2e[:],
                                                ALU.subtract)
                        nc.gpsimd.tensor_tensor(
                            dst_t[:, 2048 + 512 * c:2048 + 512 * (c + 1)],
                            t1b[:], p3n[:], ALU.add)

                # depthwise pe conv on GpSimd in the shuffled layout,
                # accumulating into att_out; center tap first (overwrite)
                def pe_conv(cc, g):
                    vv = v_sb[cc][:].rearrange("p (b r q) -> p b r q",
                                               b=2, r=64, q=32)
                    ao = att_out[cc][:].rearrange("p (b r q) -> p b r q",
                                                  b=2, r=64, q=32)
                    g0, g1 = 32 * g, 32 * g + 32
                    for b in range(2):
                        nc.vector.tensor_scalar_mul(
                            ao[:, b:b + 1, g0:g1, :],
                            vv[:, b:b + 1, g0:g1, :], pew[:, cc, 4:5])
                    for s in (0, 1, 2, 3, 5, 6, 7, 8):
                        dy, dx = s // 3 - 1, s % 3 - 1
                        r_lo = max(g0, -dy)
                        r_hi = min(g1, 64 - dy)
                        for b in range(2):
                            if dx == 0:
                                sb, q_lo, q_hi, sq = b, 0, 32, 0
                            elif dx == 1:
                                # src col 2q+b+1
                                if b == 0:
                                    sb, q_lo, q_hi, sq = 1, 0, 32, 0
                                else:
                                    sb, q_lo, q_hi, sq = 0, 0, 31, 1
                            else:
                                # src col 2q+b-1
                                if b == 1:
                                    sb, q_lo, q_hi, sq = 0, 0, 32, 0
                                else:
                                    sb, q_lo, q_hi, sq = 1, 1, 32, -1
                            dst = ao[:, b:b + 1, r_lo:r_hi, q_lo:q_hi]
                            src = vv[:, sb:sb + 1, r_lo + dy:r_hi + dy,
                                     q_lo + sq:q_hi + sq]
                            nc.vector.scalar_tensor_tensor(
                                dst, src, pew[:, cc, s:s + 1], dst,
                                ALU.mult, ALU.add)

                # v first
                conv_group(4)
                conv_group(5)

                # v^T via 4 batched XBAR DMA transposes + copies into the
                # interleaved [64ch | ones | 64ch] layout (on ACT engine)
                for cc in range(2):
                    for half in range(2):
                        vt_stg = vtt.tile([128, 32, 64], bf16, tag="vstg",
                                          name="vstg")
                        nc.sync.dma_start_transpose(
                            vt_stg[:],
                            v_sb[cc][64 * half:64 * (half + 1), :])
                        nc.scalar.copy(
                            vts_t[:, :, 130 * cc + 66 * half:
                                  130 * cc + 66 * half + 64],
                            vt_stg[:])

                pe_conv(0, 0)
                conv_group(0)
                pe_conv(0, 1)
                conv_group(1)
                pe_conv(1, 0)

                # pooling in shuffled layout: two-step reduce
                # tmp[b, by, rr, bx] = sum_q4 ; a_sum[by, bx] = sum_(b, rr)
                for cc in range(2):
                    qsrc = q_sb[cc]
                    tmp = stp.tile([128, 2, 8, 8, 8], f32, tag="pool",
                                   name="pool", bufs=2)
                    for b in range(2):
                        qv = qsrc[:, 2048 * b:2048 * (b + 1)].rearrange(
                            "p (by rr bx qq) -> p by rr bx qq",
                            by=8, rr=8, bx=8, qq=4)
                        nc.vector.tensor_reduce(
                            tmp[:, b], qv, AX.X, ALU.add)
                    tv = tmp[:].rearrange("p b by rr bx -> p by bx b rr")
                    nc.vector.tensor_reduce(a_sum[cc], tv, AX.XY, ALU.add)
                nc.vector.memset(abd_t[:], 0.0)
                for cc in range(2):
                    for j in range(4):
                        nc.vector.tensor_copy(
                            abd[cc][32 * j:32 * j + 32, 64 * j:64 * j + 64],
                            a_sum[cc][32 * j:32 * j + 32, :])

                conv_group(2)
                pe_conv(1, 1)
                conv_group(3)

            # ---- stage 1 ----
            with ExitStack() as ph:
                st_ps = ph.enter_context(
                    tc.tile_pool(name="stps", bufs=4, space="PSUM"))
                at_ps = ph.enter_context(
                    tc.tile_pool(name="atps", bufs=4, space="PSUM"))
                etp = ph.enter_context(tc.tile_pool(name="etp", bufs=1))
                attn_ps = [at_ps.tile([128, 66], f32, tag="at", name="at")
                           for _ in range(4)]
                for chp in range(16):   # px-chunk pairs
                    for cc in range(2):
                        sp = st_ps.tile([128, 512], f32, tag="st",
                                        name="stt")
                        for u in range(2):
                            ch = 2 * chp + u
                            nc.tensor.matmul(
                                sp[:, 256 * u:256 * (u + 1)],
                                k_sb[cc][:, 128 * ch:128 * (ch + 1)],
                                abd[cc], start=True, stop=True,
                                skip_group_check=True)
                        et = etp.tile([128, 512], bf16, tag="et",
                                      name="et", bufs=6)
                        nc.scalar.activation(et[:], sp[:], AF.Exp,
                                             scale=SCALE)
                        for u in range(2):
                            ch = 2 * chp + u
                            for half in range(2):
                                hp = 2 * cc + half
                                rhs = vts_t[:, ch, 130 * cc + 64 * half:
                                            130 * cc + 64 * half + 66]
                                nc.tensor.matmul(
                                    attn_ps[hp][:],
                                    et[:, 256 * u + 128 * half:
                                       256 * u + 128 * (half + 1)],
                                    rhs, start=(ch == 0), stop=(ch == 31))

                for hp in range(4):
                    half = hp % 2
                    zc = 64 if half == 0 else 0
                    och = 0 if half == 0 else 2
                    r1 = etp.tile([128, 1], f32, tag="r1", name="r1", bufs=4)
                    nc.vector.reciprocal(r1[:], attn_ps[hp][:, zc:zc + 1])
                    nc.vector.memset(attnZ[hp], 0.0)
                    nc.vector.tensor_scalar_mul(
                        attnZ[hp][0:64, 0:32],
                        attn_ps[hp][0:64, och:och + 32], r1[0:64, :])
                    nc.vector.tensor_scalar_mul(
                        attnZ[hp][64:128, 32:64],
                        attn_ps[hp][64:128, och + 32:och + 64],
                        r1[64:128, :])

            # ---- stage 2 + proj ----
            with ExitStack() as ph:
                s2sb = ph.enter_context(tc.tile_pool(name="s2sb", bufs=6))
                osb = ph.enter_context(tc.tile_pool(name="osb", bufs=3))
                s2_ps = ph.enter_context(
                    tc.tile_pool(name="s2ps", bufs=2, space="PSUM"))
                g_ps = ph.enter_context(
                    tc.tile_pool(name="gps", bufs=2, space="PSUM"))
                z_ps = ph.enter_context(
                    tc.tile_pool(name="zps", bufs=2, space="PSUM"))
                pr_ps = ph.enter_context(
                    tc.tile_pool(name="prps", bufs=2, space="PSUM"))

                def s2_chunk(nt):
                    for cc in range(2):
                        gp = g_ps.tile([128, 512], f32, tag="g", name="gt")
                        zp = z_ps.tile([128, 512], f32, tag="z", name="zt")
                        for half in range(2):
                            hp = 2 * cc + half
                            sp = s2_ps.tile([128, 512], f32, tag="s2",
                                            name="s2t")
                            nc.tensor.matmul(
                                sp[:],
                                abd_t[:, 256 * cc + 128 * half:
                                      256 * cc + 128 * (half + 1)],
                                q_sb[cc][:, 512 * nt:512 * (nt + 1)],
                                start=True, stop=True)
                            e2 = s2sb.tile([128, 512], bf16, tag="e2",
                                           name="e2")
                            nc.scalar.activation(e2[:], sp[:], AF.Exp,
                                                 scale=SCALE)
                            nc.tensor.matmul(
                                gp[64 * half:64 * half + 64, :],
                                attnZ[hp], e2[:], start=True, stop=True,
                                skip_group_check=True)
                            nc.tensor.matmul(
                                zp[64 * half:64 * half + 64, :],
                                hsel[:], e2[:], start=True, stop=True,
                                skip_group_check=True)
                        rb = s2sb.tile([128, 512], f32, tag="rb", name="rbt")
                        nc.vector.reciprocal_approx_fast(rb[:], zp[:])
                        tsc = s2sb.tile([128, 512], bf16, tag="ts",
                                        name="tsc")
                        nc.vector.tensor_tensor(tsc[:], gp[:], rb[:],
                                                ALU.mult)
                        sl = att_out[cc][:, 512 * nt:512 * (nt + 1)]
                        nc.gpsimd.tensor_tensor(sl, tsc[:], sl, ALU.add)

                # process chunk pairs (b=0, rc), (b=1, rc) then proj rc
                for rc in range(4):
                    s2_chunk(rc)          # b=0 chunk
                    s2_chunk(4 + rc)      # b=1 chunk
                    for mc in range(2):
                        ot = osb.tile([128, 16, 32, 2], bf16, tag="ot",
                                      name="ott")
                        for b in range(2):
                            pp = pr_ps.tile([128, 512], f32, tag="tp",
                                            name="prt")
                            for kc in range(2):
                                nc.tensor.matmul(
                                    pp[:],
                                    pwv[:, kc, 128 * mc:128 * (mc + 1)],
                                    att_out[kc][:, 2048 * b + 512 * rc:
                                                2048 * b + 512 * (rc + 1)],
                                    start=(kc == 0), stop=(kc == 1))
                            ppv = pp[:].rearrange("p (r q one) -> p r q one",
                                                  r=16, q=32, one=1)
                            nc.scalar.add(ot[:, :, :, b:b + 1], ppv,
                                          pb[:, mc:mc + 1])
                        nc.sync.dma_start(
                            OUT[mc, :, 1024 * rc:1024 * (rc + 1)],
                            ot[:].rearrange("p r q two -> p (r q two)"))

    nc.compile()
    return nc


def _prep_consts(qkv_w, qkv_s, qkv_b, pe_w, pe_s, pe_b, proj_w, proj_s,
                 proj_b):
    import ml_dtypes
    f = np.float32
    bf = ml_dtypes.bfloat16
    w = np.asarray(qkv_w, f).copy()          # [768, 256, 3, 3]
    dif = (w[:, :, 0, 1] + w[:, :, 1, 0] + w[:, :, 1, 1] + w[:, :, 1, 2]
           + w[:, :, 2, 1])
    w[:, :, 1, 1] -= THETA * dif
    w *= np.asarray(qkv_s, f)[:, None, None, None]

    # 1-D Winograd F(2,3) weight transform along kx:
    # What[0]=w0, What[1]=(w0+w1+w2)/2, What[2]=(w0-w1+w2)/2, What[3]=w2
    w0, w1, w2 = w[:, :, :, 0], w[:, :, :, 1], w[:, :, :, 2]
    wh = np.stack([w0, (w0 + w1 + w2) * 0.5, (w0 - w1 + w2) * 0.5, w2],
                  axis=0)                     # [4, 768, 256, 3(ky)]
    # WQW[mc, p, j, ky, kc, o'] = wh[j, 128*mc+o', 128*kc+p, ky]
    wh = wh.reshape(4, 6, 128, 2, 128, 3)     # [j, mc, o', kc, p, ky]
    wqw = np.ascontiguousarray(
        wh.transpose(1, 4, 0, 5, 3, 2)).astype(bf)  # [mc,p,j,ky,kc,o']

    bq = np.ascontiguousarray(np.asarray(qkv_b, f).reshape(6, 128).T)

    pe_wf = np.asarray(pe_w, f)[:, 0] * np.asarray(pe_s, f)[:, None, None]
    pew = np.zeros((128, 2, 9), f)
    for kc in range(2):
        for s in range(9):
            pew[:, kc, s] = pe_wf[128 * kc:128 * (kc + 1), s // 3, s % 3]

    pwm = np.asarray(proj_w, f)[:, :, 0, 0] * np.asarray(proj_s, f)[:, None]
    pw = np.ascontiguousarray(
        pwm.T.reshape(2, 128, 256).transpose(1, 0, 2).reshape(
            128, 512)).astype(bf)
    pbv = np.asarray(proj_b, f) + pwm @ np.asarray(pe_b, f)
    pb = np.ascontiguousarray(pbv.reshape(2, 128).T)

    return dict(wqw=wqw, bq=bq, pew=pew, pw=pw, pb=pb)


def kernel(x, qkv_w, qkv_s, qkv_b, pe_w, pe_s, pe_b, proj_w, proj_s, proj_b):
    import ml_dtypes
    from concourse.bass_utils import run_bass_kernel_spmd

    if "nc" not in _cache:
        _cache["nc"] = _build()
    nc = _cache["nc"]

    consts = _prep_consts(qkv_w, qkv_s, qkv_b, pe_w, pe_s, pe_b, proj_w,
                          proj_s, proj_b)
    x = np.asarray(x, np.float32).astype(ml_dtypes.bfloat16)
    in_maps = []
    for b in range(B):
        m = dict(consts)
        m["x"] = np.ascontiguousarray(x[b].reshape(2, 128, HW))
        in_maps.append(m)

    res = run_bass_kernel_spmd(nc, in_maps, list(range(N_CORES)), trace=False)
    out = np.empty((B, C, H, W), np.float32)
    for b in range(B):
        o = np.asarray(res.results[b]["out"], np.float32).reshape(C, HW)
        out[b] = o.reshape(C, H, W)
    return out
